# revision 15
# baseline (speedup 1.0000x reference)
"""Trainium2 Bass kernel: Conv3d(32->64,k3,SAME) + InstanceNorm3d + LeakyReLU
+ single-head self-attention over N=8000 spatial positions, B=2.

Sharding: 8 cores, core = 4*b + c. Each core computes the full conv/norm/
K/V for its batch (redundantly within the 4-core batch group) and the
attention output for its 2000-query slice (d-planes [5c, 5c+5)).

All per-core differences are host-side data (SPMD program is identical):
  - xg: (dz,ci)-packed zero-padded input planes for the full conv
  - xq: per-core 5-output-plane padded input for the Q-block conv
  - weights pre-laid-out on host (bias rows folded in via ones-row trick)

Attention is computed in transposed-score space: sT[k, q] = K^T Q so the
softmax denominator arrives via an ones-row appended to V^T (row 64 of the
PV accumulator), avoiding any cross-partition reductions.
"""

import sys

sys.path.insert(0, "/opt/trn_rl_repo")

import numpy as np

B, CIN, COUT, D, H, W = 2, 32, 64, 20, 20, 20

# Input-blob column layout (floats per partition row)
OFF_XG = 0            # [96, 9680]  (dz,ci)-packed padded conv input
OFF_XQ = 9680         # [96, 2420]  per-core Q-block conv input
OFF_WC = 12100        # [96, 576]   conv weights, (dy,dx)-major
OFF_WK = 12676        # [64, 64]    wk^T
OFF_WV = 12740        # [64, 64]
OFF_WQ = 12804        # [64, 64]
OFF_G = 12868         # [64, 1] gamma
OFF_B = 12869         # [64, 1] beta
OFF_BK = 12870        # [64, 1]
OFF_BV = 12871        # [64, 1]
OFF_BQ = 12872        # [64, 1]
BLOB_F = 12876
N = D * H * W            # 8000
NCORES = 8
QS = N // 4              # 2000 queries per core
EPS = 1e-5
NEG_SLOPE = 0.01

_cache = {}


def _build_program():
    from contextlib import ExitStack

    import concourse.bass as bass
    import concourse.tile as tile_mod
    from concourse import mybir

    F32 = mybir.dt.float32
    F32R = mybir.dt.float32r
    AF = mybir.ActivationFunctionType
    ALU = mybir.AluOpType

    nc = bass.Bass()

    # All inputs ride in one blob so every consumer waits on a single DMA
    # semaphore (matmuls can carry only one semaphore wait).
    BLOB = nc.declare_dram_parameter("blob", [128, BLOB_F], F32R, isOutput=False)
    OUT = nc.declare_dram_parameter("out", [64, QS], F32, isOutput=True)

    # DRAM bounce buffers for the tiny rowsum reshape (partition <-> free).
    RS_SC = nc.dram_tensor("rs_sc", [QS], F32)
    RR_SC = nc.dram_tensor("rr_sc", [QS], F32)

    def r(ap):
        return ap.bitcast(F32R)

    with tile_mod.TileContext(nc) as tc, ExitStack() as stack:
        consts = stack.enter_context(tc.tile_pool(name="consts", bufs=1))
        main = stack.enter_context(tc.tile_pool(name="main", bufs=1))
        # One PSUM pool for the whole kernel (no mid-kernel pool releases —
        # release waits would push matmuls over their one-wait budget).
        # Tag bank budget: convps 2 + actps 4 + pv0 1 + pv1 1 = 8 banks.
        psum = stack.enter_context(tc.tile_pool(name="psum", bufs=1, space="PSUM"))
        normp = stack.enter_context(tc.tile_pool(name="normp", bufs=2))
        etp = stack.enter_context(tc.tile_pool(name="etp", bufs=3))

        eps_t = consts.tile([64, 1], F32, name="eps_t")
        nc.vector.memset(eps_t, EPS)

        # Persistent tensors. Y doubles as F (normalized in place).
        Y = main.tile([64, 20, 400], F32, name="Y")
        Yq = main.tile([64, 5, 400], F32, name="Yq")
        K = main.tile([64, 8000], F32, name="K")
        Q = main.tile([64, QS], F32, name="Q")
        VT = main.tile([128, 63, 65], F32, name="VT")
        out_sb = main.tile([64, QS], F32, name="out_sb")
        rs = main.tile([1, QS], F32, name="rs")

        blob, free_blob = tc.tile([128, BLOB_F], F32R, name="blob")
        nc.sync.dma_start(out=blob, in_=BLOB[:, :])

        Xg = blob[0:96, OFF_XG : OFF_XG + 9680].rearrange(
            "p (a b c) -> p a b c", a=20, b=22, c=22)
        Xq = blob[0:96, OFF_XQ : OFF_XQ + 2420].rearrange(
            "p (a b c) -> p a b c", a=5, b=22, c=22)
        WcV = blob[0:96, OFF_WC : OFF_WC + 576].rearrange("p (t c) -> p t c", t=9)

        # Small weights are copied out so the blob can be freed after conv.
        Wkt = consts.tile([64, 64], F32R, name="Wkt")
        nc.vector.tensor_copy(out=Wkt, in_=blob[0:64, OFF_WK : OFF_WK + 64])
        Wvt = consts.tile([64, 64], F32R, name="Wvt")
        nc.vector.tensor_copy(out=Wvt, in_=blob[0:64, OFF_WV : OFF_WV + 64])
        Wqt = consts.tile([64, 64], F32R, name="Wqt")
        nc.vector.tensor_copy(out=Wqt, in_=blob[0:64, OFF_WQ : OFF_WQ + 64])
        bk_t = consts.tile([64, 1], F32, name="bk_t")
        nc.vector.tensor_copy(out=bk_t, in_=blob[0:64, OFF_BK : OFF_BK + 1].bitcast(F32))
        bv_t = consts.tile([64, 1], F32, name="bv_t")
        nc.vector.tensor_copy(out=bv_t, in_=blob[0:64, OFF_BV : OFF_BV + 1].bitcast(F32))
        bq_t = consts.tile([64, 1], F32, name="bq_t")
        nc.vector.tensor_copy(out=bq_t, in_=blob[0:64, OFF_BQ : OFF_BQ + 1].bitcast(F32))
        gam = consts.tile([64, 1], F32, name="gam")
        nc.vector.tensor_copy(out=gam, in_=blob[0:64, OFF_G : OFF_G + 1].bitcast(F32))
        bet = consts.tile([64, 1], F32, name="bet")
        nc.vector.tensor_copy(out=bet, in_=blob[0:64, OFF_B : OFF_B + 1].bitcast(F32))

        # ---- Conv3d as 9 (dy,dx)-tap matmuls over (dz,ci)-packed input ----
        for d in range(20):
            ps = psum.tile([128, 512], F32, name="ps_conv", tag="convps", bufs=2)
            for t in range(9):
                dy, dx = divmod(t, 3)
                rhs = Xg[:, d, dy : dy + 20, dx : dx + 20]
                nc.tensor.matmul(
                    ps[0:64, :400], lhsT=r(WcV[:, t, :]), rhs=r(rhs),
                    start=(t == 0), stop=(t == 8),
                )
            nc.vector.tensor_copy(out=r(Y[0:64, d, :]), in_=ps[0:64, :400])
        for lo in range(5):
            ps = psum.tile([128, 512], F32, name="ps_convq", tag="convps", bufs=2)
            for t in range(9):
                dy, dx = divmod(t, 3)
                rhs = Xq[:, lo, dy : dy + 20, dx : dx + 20]
                nc.tensor.matmul(
                    ps[0:64, :400], lhsT=r(WcV[:, t, :]), rhs=r(rhs),
                    start=(t == 0), stop=(t == 8),
                )
            nc.vector.tensor_copy(out=r(Yq[0:64, lo, :]), in_=ps[0:64, :400])
        free_blob()

        Yf = Y.rearrange("p a b -> p (a b)")     # [64, 8000]
        Yqf = Yq.rearrange("p a b -> p (a b)")   # [64, 2000]

        # ---- InstanceNorm stats over the full spatial extent ----
        stats = main.tile([64, 16, 6], F32, name="stats")
        for j in range(16):
            nc.vector.bn_stats(
                out=stats[:, j, :], in_=Yf[0:64, j * 500 : (j + 1) * 500]
            )
        mv = main.tile([64, 2], F32, name="mv")
        nc.vector.bn_aggr(out=mv, in_=stats)
        std = main.tile([64, 1], F32, name="std")
        nc.scalar.activation(std, mv[:, 1:2], AF.Sqrt, bias=eps_t, scale=1.0)
        rstd = main.tile([64, 1], F32, name="rstd")
        nc.vector.reciprocal(rstd, std)
        scal = main.tile([64, 1], F32, name="scal")
        nc.vector.tensor_mul(scal, rstd, gam)
        tmp_ms = main.tile([64, 1], F32, name="tmp_ms")
        nc.vector.tensor_mul(tmp_ms, mv[:, 0:1], scal)
        shift = main.tile([64, 1], F32, name="shift")
        nc.vector.tensor_sub(shift, bet, tmp_ms)

        # ---- Normalize + LeakyReLU in place; set the ones rows (all DVE) ----
        nc.vector.tensor_scalar(
            out=r(Yf[0:64, :]), in0=Yf[0:64, :], scalar1=scal, scalar2=shift,
            op0=ALU.mult, op1=ALU.add,
        )
        nc.vector.scalar_tensor_tensor(
            out=r(Yf[0:64, :]), in0=Yf[0:64, :], scalar=NEG_SLOPE, in1=Yf[0:64, :],
            op0=ALU.mult, op1=ALU.max,
        )

        nc.vector.tensor_scalar(
            out=r(Yqf[0:64, :]), in0=Yqf[0:64, :], scalar1=scal, scalar2=shift,
            op0=ALU.mult, op1=ALU.add,
        )
        nc.vector.scalar_tensor_tensor(
            out=r(Yqf[0:64, :]), in0=Yqf[0:64, :], scalar=NEG_SLOPE, in1=Yqf[0:64, :],
            op0=ALU.mult, op1=ALU.max,
        )

        # ---- Projections: K [64,8000], Q [64,2000], V^T [128, 63, 65] ----
        # K/Q psum copies stay on DVE (merge with the Yf writer waits);
        # VT psum copies live on ScalarE so the PV matmuls (which also read
        # the ACT-produced exp tiles) carry a single-engine wait.
        for j in range(16):
            ps = psum.tile([128, 1024], F32, name="ps_k", tag="actps", bufs=2)
            nc.tensor.matmul(
                ps[0:64, :500], lhsT=r(Wkt), rhs=r(Yf[:, j * 500 : (j + 1) * 500]),
                start=True, stop=True,
            )
            nc.vector.tensor_scalar_add(
                out=r(K[:, j * 500 : (j + 1) * 500]), in0=ps[0:64, :500], scalar1=bk_t
            )
        for j in range(4):
            ps = psum.tile([128, 1024], F32, name="ps_q", tag="actps", bufs=2)
            nc.tensor.matmul(
                ps[0:64, :500], lhsT=r(Wqt), rhs=r(Yqf[:, j * 500 : (j + 1) * 500]),
                start=True, stop=True,
            )
            nc.vector.tensor_scalar_add(
                out=r(Q[:, j * 500 : (j + 1) * 500]), in0=ps[0:64, :500], scalar1=bq_t
            )
        nc.scalar.activation(r(VT[:, :, 64:65]), VT[:, :, 64:65], AF.Copy,
                             bias=1.0, scale=0.0)
        for kc in range(63):
            kp = 128 if kc < 62 else 64
            ps = psum.tile([128, 512], F32, name="ps_vt", tag="convps", bufs=2)
            nc.tensor.matmul(
                ps[0:kp, 0:64], lhsT=r(Yf[:, kc * 128 : kc * 128 + kp]), rhs=r(Wvt),
                start=True, stop=True,
            )
            nc.scalar.copy(out=r(VT[0:kp, kc, 0:64]), in_=ps[0:kp, 0:64])

        # ---- Attention: two passes over 1000-query halves ----
        # sT = K^T Q per 128-key chunk -> exp on ACT -> PV accumulate, with
        # the softmax denominator arriving via VT's ones row (row 64).
        for p in range(2):
            qoff = p * 1000
            subs = ((0, 512), (512, 488))
            pva = psum.tile([65, 512], F32, name=f"pva{p}", tag="pv0", bufs=1)
            pvb = psum.tile([65, 512], F32, name=f"pvb{p}", tag="pv1", bufs=1)
            pv = {0: pva, 512: pvb}
            for kc in range(63):
                kp = 128 if kc < 62 else 64
                Ksl = K[:, kc * 128 : kc * 128 + kp]
                ps = psum.tile([128, 1024], F32, name="ps_s", tag="actps", bufs=2)
                for o, w in subs:
                    nc.tensor.matmul(
                        ps[0:kp, o : o + w], lhsT=r(Ksl),
                        rhs=r(Q[:, qoff + o : qoff + o + w]),
                        start=True, stop=True,
                    )
                ET = etp.tile([128, 1024], F32, name="ET", tag="ET")
                nc.scalar.activation(r(ET[0:kp, 0:1000]), ps[0:kp, 0:1000], AF.Exp)
                for o, w in subs:
                    nc.tensor.matmul(
                        pv[o][:, 0:w], lhsT=r(VT[0:kp, kc, :]),
                        rhs=r(ET[0:kp, o : o + w]),
                        start=(kc == 0), stop=(kc == 62),
                    )

            # Per-pass normalization by the rowsum (row 64 of PV psum).
            # PV psum is drained through ACT copies so the next pass's PV
            # matmuls only ever wait on ACT.
            pvsb = {}
            for o, w in subs:
                t = normp.tile([65, 512], F32, name=f"pvsb{p}_{o}", tag="pvsb")
                nc.scalar.copy(out=t[:, 0:w], in_=pv[o][:, 0:w])
                pvsb[o] = t
                nc.vector.tensor_copy(
                    out=rs[:, qoff + o : qoff + o + w], in_=t[64:65, 0:w]
                )
            nc.sync.dma_start(
                out=RS_SC[qoff : qoff + 1000], in_=rs[:, qoff : qoff + 1000]
            )
            rsp = normp.tile([125, 8], F32, name=f"rsp{p}", tag="rsp")
            nc.sync.dma_start(
                out=rsp,
                in_=RS_SC[qoff : qoff + 1000].rearrange("(a b) -> a b", b=8),
            )
            rrp = normp.tile([125, 8], F32, name=f"rrp{p}", tag="rrp")
            nc.vector.reciprocal(rrp, rsp)
            nc.sync.dma_start(
                out=RR_SC[qoff : qoff + 1000].rearrange("(a b) -> a b", b=8),
                in_=rrp,
            )
            rrb = normp.tile([64, 1000], F32, name=f"rrb{p}", tag="rrb")
            rr_src = RR_SC[qoff : qoff + 1000]
            rr_bcast = bass.AP(
                tensor=rr_src.tensor, offset=rr_src.offset,
                ap=[[0, 64]] + [list(x) for x in rr_src.ap],
            )
            nc.sync.dma_start(out=rrb, in_=rr_bcast)
            for o, w in subs:
                nc.vector.tensor_mul(
                    out=out_sb[:, qoff + o : qoff + o + w],
                    in0=pvsb[o][0:64, 0:w], in1=rrb[:, o : o + w],
                )
                nc.vector.tensor_scalar_add(
                    out=out_sb[:, qoff + o : qoff + o + w],
                    in0=out_sb[:, qoff + o : qoff + o + w], scalar1=bv_t,
                )
        nc.sync.dma_start(out=OUT[:, :], in_=out_sb)

    # TRN2 allows one embedded wait per instruction; split the rest into
    # EventSemaphore instructions (the bacc pass walrus expects).
    import bass_rust as _bass_rust
    _bass_rust.generate_event_semaphores(nc)
    return nc


def _host_prep(x, conv_w, gamma, beta, wq, bq, wk, bk, wv, bv):
    """Build the 8 per-core input maps."""
    f32 = np.float32
    x = np.ascontiguousarray(x, dtype=f32)
    x_pad = np.zeros((B, CIN, 22, 22, 22), dtype=f32)
    x_pad[:, :, 1:21, 1:21, 1:21] = x
    # xg[b, 32*dz+ci, a, hp, wp] = x_pad[b, ci, a+dz, hp, wp]
    xg = np.empty((B, 3, CIN, 20, 22, 22), dtype=f32)
    for dz in range(3):
        xg[:, dz] = x_pad[:, :, dz : dz + 20]
    xg = np.ascontiguousarray(xg.reshape(B, 96, 20, 22, 22))

    # wg[32*dz+ci, 3*dy+dx, co] = conv_w[co, ci, dz, dy, dx]
    wg = np.ascontiguousarray(
        conv_w.astype(f32).transpose(2, 1, 3, 4, 0).reshape(96, 9, 64)
    )

    wkt = np.ascontiguousarray(wk.astype(f32).T)
    wvt = np.ascontiguousarray(wv.astype(f32).T)
    wqt = np.ascontiguousarray(wq.astype(f32).T)
    gam = np.ascontiguousarray(gamma.astype(f32).reshape(64, 1))
    bet = np.ascontiguousarray(beta.astype(f32).reshape(64, 1))

    in_maps = []
    for core in range(NCORES):
        b, c = divmod(core, 4)
        blob = np.zeros((128, BLOB_F), dtype=f32)
        blob[0:96, OFF_XG : OFF_XG + 9680] = xg[b].reshape(96, 9680)
        blob[0:96, OFF_XQ : OFF_XQ + 2420] = xg[b][:, 5 * c : 5 * c + 5].reshape(96, 2420)
        blob[0:96, OFF_WC : OFF_WC + 576] = wg.reshape(96, 576)
        blob[0:64, OFF_WK : OFF_WK + 64] = wkt
        blob[0:64, OFF_WV : OFF_WV + 64] = wvt
        blob[0:64, OFF_WQ : OFF_WQ + 64] = wqt
        blob[0:64, OFF_G] = gam[:, 0]
        blob[0:64, OFF_B] = bet[:, 0]
        blob[0:64, OFF_BK] = bk.astype(f32)
        blob[0:64, OFF_BV] = bv.astype(f32)
        blob[0:64, OFF_BQ] = bq.astype(f32)
        in_maps.append({"blob": blob})
    return in_maps


def _get_nc():
    if "nc" not in _cache:
        _cache["nc"] = _build_program()
    return _cache["nc"]


def run(inputs, trace=False):
    """Run on the 8 NeuronCores. Returns (full_output, exec_time_ns|None)."""
    from concourse.bass_utils import run_bass_kernel_spmd

    in_maps = _host_prep(
        inputs["x"], inputs["conv_w"], inputs["gamma"], inputs["beta"],
        inputs["wq"], inputs["bq"], inputs["wk"], inputs["bk"],
        inputs["wv"], inputs["bv"],
    )
    nc = _get_nc()
    res = run_bass_kernel_spmd(
        nc, in_maps, list(range(NCORES)), trace=trace,
        trace_cores=list(range(NCORES)) if trace else None,
    )
    out = np.empty((B, COUT, N), dtype=np.float32)
    for core in range(NCORES):
        b, c = divmod(core, 4)
        out[b, :, c * QS : (c + 1) * QS] = res.results[core]["out"]
    return out.reshape(B, COUT, D, H, W), res.exec_time_ns


def kernel(**inputs):
    out, _ = run(inputs)
    return out


def simulate_core(inputs, core=0):
    """Run one core through the bass interpreter (no hardware). Debug aid."""
    from concourse import bass_interp

    in_maps = _host_prep(
        inputs["x"], inputs["conv_w"], inputs["gamma"], inputs["beta"],
        inputs["wq"], inputs["bq"], inputs["wk"], inputs["bk"],
        inputs["wv"], inputs["bv"],
    )
    nc = _get_nc()
    sim = bass_interp.MultiCoreSim(nc, 1)
    for name, arr in in_maps[core].items():
        sim.cores[0].tensor(name)[:] = arr
    sim.simulate()
    return np.array(sim.cores[0].tensor("out"))
